# revision 1
# baseline (speedup 1.0000x reference)
"""MoE gated 3x3 conv (eval path) on 8 trn2 NeuronCores.

Strategy:
- Routing (tiny: [16,64]@[64,16] -> softmax -> top-4 gates) and the scalar
  aux loss are computed on host; the gates determine a per-sample merged
  conv weight  Wc[b] = sum_e gates[b,e] * W[e]  (conv is linear in the
  weights, and zero-gate experts contribute nothing), so the device does
  exactly one 3x3 conv per sample instead of num_experts of them.
- Data parallel over batch: 2 samples per core. Sample 0 lives on SBUF
  partitions 0-63, sample 1 on partitions 64-127, which makes the two
  per-sample matmul chains row-tiled (tile_position (0,0) / (64,0)) so the
  PE array runs both concurrently.
- The conv is 9 shift-matmuls (taps) accumulating in PSUM over a zero-padded
  [Cin, 66, 66] image view, 8 output rows (N=512) per accumulation group.
"""

import numpy as np

import concourse.bacc as bacc
import concourse.tile as tile
from concourse import mybir
from concourse.bass_utils import run_bass_kernel_spmd

N_CORES = 8
B, CIN, COUT, E = 16, 64, 64, 16
H = W_SP = 64
KTOP = 4
PH = PW = H + 2  # zero-padded image
RB = 8           # output rows per PSUM chunk (RB * W_SP = 512 = fp32 matmul max N)
NCHUNK = H // RB
F32 = mybir.dt.float32

_PROGRAM_CACHE = {}


def _routing_gates(x, w_gate):
    """Eval-path gates, mirroring the reference: softmax over clean logits,
    top-4 renormalized. [B, E] float32."""
    gate_x = x.reshape(B, CIN, H * W_SP).mean(axis=2)      # [B, Cin]
    logits = gate_x.astype(np.float32) @ w_gate            # [B, E]
    m = logits.max(axis=1, keepdims=True)
    ex = np.exp(logits - m)
    sm = ex / ex.sum(axis=1, keepdims=True)
    idx = np.argsort(-sm, axis=1, kind="stable")[:, :KTOP]
    vals = np.take_along_axis(sm, idx, axis=1)
    gk = vals / (vals.sum(axis=1, keepdims=True) + 1e-6)
    gates = np.zeros((B, E), np.float32)
    np.put_along_axis(gates, idx, gk.astype(np.float32), axis=1)
    return gates


def _aux_loss(gates):
    load = (gates > 0).sum(axis=0).astype(np.float32)
    importance = gates.sum(axis=0).astype(np.float32)

    def cv_sq(v):
        return v.var(ddof=1) / (v.mean() ** 2 + 1e-10)

    return np.float32((cv_sq(importance) + cv_sq(load)) * 0.01)


def _build_program():
    nc = bacc.Bacc("TRN2", target_bir_lowering=False, debug=False,
                   num_devices=N_CORES)
    x2 = nc.dram_tensor("x2", [2, CIN, H, W_SP], F32, kind="ExternalInput").ap()
    wct = nc.dram_tensor("wct", [128, 9 * COUT], F32, kind="ExternalInput").ap()
    bias2 = nc.dram_tensor("bias2", [COUT, 2], F32, kind="ExternalInput").ap()
    y2 = nc.dram_tensor("y2", [2, COUT, H, W_SP], F32, kind="ExternalOutput").ap()

    with tile.TileContext(nc) as tc:
        with tc.tile_pool(name="xs", bufs=1) as xpool, \
             tc.tile_pool(name="w", bufs=1) as wpool, \
             tc.tile_pool(name="out", bufs=3) as opool, \
             tc.tile_pool(name="ps", bufs=2, space="PSUM") as pspool:
            xs = xpool.tile([128, PH * PW], F32)
            v = xs.rearrange("p (h w) -> p h w", w=PW)
            # Only the 1-wide border needs to be zero.
            nc.vector.memset(v[:, 0, :], 0.0)
            nc.vector.memset(v[:, PH - 1, :], 0.0)
            nc.vector.memset(v[:, :, 0], 0.0)
            nc.vector.memset(v[:, :, PW - 1], 0.0)
            nc.sync.dma_start(out=v[0:64, 1:H + 1, 1:W_SP + 1], in_=x2[0])
            nc.sync.dma_start(out=v[64:128, 1:H + 1, 1:W_SP + 1], in_=x2[1])

            wsb = wpool.tile([128, 9 * COUT], F32)
            nc.sync.dma_start(out=wsb, in_=wct)
            w3 = wsb.rearrange("p (t c) -> p t c", t=9)
            bsb = wpool.tile([COUT, 2], F32)
            nc.sync.dma_start(out=bsb, in_=bias2)

            for r in range(NCHUNK):
                psA = pspool.tile([COUT, RB * W_SP], F32, tag="psA")
                psB = pspool.tile([COUT, RB * W_SP], F32, tag="psB")
                for t in range(9):
                    dy, dx = divmod(t, 3)
                    rA = v[0:64, RB * r + dy: RB * r + dy + RB, dx: dx + W_SP]
                    rB = v[64:128, RB * r + dy: RB * r + dy + RB, dx: dx + W_SP]
                    nc.tensor.matmul(psA, lhsT=w3[0:64, t, :], rhs=rA,
                                     start=(t == 0), stop=(t == 8))
                    nc.tensor.matmul(psB, lhsT=w3[64:128, t, :], rhs=rB,
                                     start=(t == 0), stop=(t == 8))
                oA = opool.tile([COUT, RB * W_SP], F32, tag="oA")
                oB = opool.tile([COUT, RB * W_SP], F32, tag="oB")
                nc.vector.tensor_scalar_add(oA, psA, bsb[:, 0:1])
                nc.vector.tensor_scalar_add(oB, psB, bsb[:, 1:2])
                nc.sync.dma_start(out=y2[0, :, RB * r: RB * (r + 1), :],
                                  in_=oA.rearrange("p (h w) -> p h w", w=W_SP))
                nc.sync.dma_start(out=y2[1, :, RB * r: RB * (r + 1), :],
                                  in_=oB.rearrange("p (h w) -> p h w", w=W_SP))
    nc.compile()
    return nc


def get_program():
    if "nc" not in _PROGRAM_CACHE:
        _PROGRAM_CACHE["nc"] = _build_program()
    return _PROGRAM_CACHE["nc"]


def make_in_maps(x, Wc, bc):
    """Per-core input maps: 2 samples per core."""
    in_maps = []
    for c in range(N_CORES):
        s0, s1 = 2 * c, 2 * c + 1
        # wct[ci + 64*s, t*64 + co] = Wc[sample, co, ci, t]
        w0 = Wc[s0].reshape(COUT, CIN, 9).transpose(1, 2, 0)
        w1 = Wc[s1].reshape(COUT, CIN, 9).transpose(1, 2, 0)
        wct = np.concatenate([w0, w1], axis=0).reshape(128, 9 * COUT)
        bias2 = np.stack([bc[s0], bc[s1]], axis=1)  # [COUT, 2]
        in_maps.append({
            "x2": np.ascontiguousarray(x[s0:s1 + 1]),
            "wct": np.ascontiguousarray(wct),
            "bias2": np.ascontiguousarray(bias2),
        })
    return in_maps


def kernel(**inputs):
    x = np.asarray(inputs["x"], dtype=np.float32)
    w_gate = np.asarray(inputs["w_gate"], dtype=np.float32)
    W = np.asarray(inputs["W"], dtype=np.float32)
    b = np.asarray(inputs["b"], dtype=np.float32)
    # train is eval-only in the reference; the noise branch never runs.

    gates = _routing_gates(x, w_gate)
    loss = _aux_loss(gates)
    Wc = np.tensordot(gates, W.reshape(E, -1), axes=(1, 0)) \
        .reshape(B, COUT, CIN, 3, 3)
    bc = gates @ b  # [B, COUT]

    nc = get_program()
    res = run_bass_kernel_spmd(nc, make_in_maps(x, Wc, bc),
                               core_ids=list(range(N_CORES)))
    y = np.concatenate([res.results[c]["y2"] for c in range(N_CORES)], axis=0)
    return (y, loss)


# revision 2
# speedup vs baseline: 2.1696x; 2.1696x over previous
"""MoE gated 3x3 conv (eval path) on 8 trn2 NeuronCores.

Strategy:
- Routing (tiny: [16,64]@[64,16] -> softmax -> top-4 gates) and the scalar
  aux loss are computed on host; the gates determine a per-sample merged
  conv weight  Wc[b] = sum_e gates[b,e] * W[e]  (conv is linear in the
  weights, and zero-gate experts contribute nothing), so the device does
  exactly one 3x3 conv per sample instead of num_experts of them.
- Data parallel over batch: 2 samples per core. Sample 0 lives on SBUF
  partitions 0-63, sample 1 on partitions 64-127, so the two per-sample
  matmul chains are row-tiled (tile_position (0,0)/(64,0)) and the PE
  runs both concurrently.
- Width-65 padded image layout: row i of the padded image is
  [0, x[i-1, 0..63]], with zero rows above and below. A single zero
  column between consecutive rows serves as BOTH the right pad of row i
  and the left pad of row i+1, so every conv tap (dy, dx) is a pure flat
  shift by dy*65+dx and the matmul moving operand is fully contiguous.
- The conv is 9 shift-matmuls accumulating in PSUM, 7 output rows
  (N = 7*65+1 = 456) per accumulation group; 10 chunks cover H=64 (the
  last chunk overlaps by 6 rows; only its last row is stored).
- Matmuls run as float32r (TF32-like single-pass PE mode, ~1.5e-4
  scale-relative output error); PSUM accumulation stays fp32. Outputs are
  written to a [10, 456] staging layout (junk columns included) and
  compacted on host.
"""

import numpy as np

import concourse.bacc as bacc
import concourse.tile as tile
from concourse import mybir
from concourse.bass_utils import run_bass_kernel_spmd

N_CORES = 8
B, CIN, COUT, E = 16, 64, 64, 16
H = W_SP = 64
KTOP = 4
PW = W_SP + 1          # 65: one shared zero column per row
NROW = H + 3           # 67 padded rows (top zero, bottom zero + tap overrun)
XFLAT = NROW * PW      # 4355
RB = 7                 # output rows per PSUM chunk
NMM = RB * PW + 1      # 456: fp32r needs an even moving size
CHUNK_STARTS = (0, 7, 14, 21, 28, 35, 42, 49, 56, 57)
F32 = mybir.dt.float32
MM_DT = mybir.dt.float32r

_PROGRAM_CACHE = {}


def _routing_gates(x, w_gate):
    """Eval-path gates, mirroring the reference: softmax over clean logits,
    top-4 renormalized. [B, E] float32."""
    gate_x = x.reshape(B, CIN, H * W_SP).mean(axis=2)      # [B, Cin]
    logits = gate_x.astype(np.float32) @ w_gate            # [B, E]
    m = logits.max(axis=1, keepdims=True)
    ex = np.exp(logits - m)
    sm = ex / ex.sum(axis=1, keepdims=True)
    idx = np.argsort(-sm, axis=1, kind="stable")[:, :KTOP]
    vals = np.take_along_axis(sm, idx, axis=1)
    gk = vals / (vals.sum(axis=1, keepdims=True) + 1e-6)
    gates = np.zeros((B, E), np.float32)
    np.put_along_axis(gates, idx, gk.astype(np.float32), axis=1)
    return gates


def _aux_loss(gates):
    load = (gates > 0).sum(axis=0).astype(np.float32)
    importance = gates.sum(axis=0).astype(np.float32)

    def cv_sq(v):
        return v.var(ddof=1) / (v.mean() ** 2 + 1e-10)

    return np.float32((cv_sq(importance) + cv_sq(load)) * 0.01)


def _build_program():
    nc = bacc.Bacc("TRN2", target_bir_lowering=False, debug=False,
                   num_devices=N_CORES)
    xp = nc.dram_tensor("xp", [128, XFLAT], MM_DT, kind="ExternalInput").ap()
    wct = nc.dram_tensor("wct", [128, 9 * COUT], MM_DT,
                         kind="ExternalInput").ap()
    bias2 = nc.dram_tensor("bias2", [COUT, 2], F32, kind="ExternalInput").ap()
    y2p = nc.dram_tensor("y2p", [2, COUT, len(CHUNK_STARTS), NMM], F32,
                         kind="ExternalOutput").ap()

    with tile.TileContext(nc) as tc:
        with tc.tile_pool(name="xs", bufs=1) as xpool, \
             tc.tile_pool(name="w", bufs=1) as wpool, \
             tc.tile_pool(name="out", bufs=3) as opool, \
             tc.tile_pool(name="ps", bufs=2, space="PSUM") as pspool:
            xs = xpool.tile([128, XFLAT], MM_DT)
            nc.sync.dma_start(out=xs, in_=xp)
            wsb = wpool.tile([128, 9 * COUT], MM_DT)
            nc.sync.dma_start(out=wsb, in_=wct)
            w3 = wsb.rearrange("p (t c) -> p t c", t=9)
            bsb = wpool.tile([COUT, 2], F32)
            nc.sync.dma_start(out=bsb, in_=bias2)

            for g, r0 in enumerate(CHUNK_STARTS):
                psA = pspool.tile([COUT, NMM], F32, tag="psA")
                psB = pspool.tile([COUT, NMM], F32, tag="psB")
                for t in range(9):
                    dy, dx = divmod(t, 3)
                    o = (r0 + dy) * PW + dx
                    nc.tensor.matmul(psA, lhsT=w3[0:64, t, :],
                                     rhs=xs[0:64, o:o + NMM],
                                     start=(t == 0), stop=(t == 8))
                    nc.tensor.matmul(psB, lhsT=w3[64:128, t, :],
                                     rhs=xs[64:128, o:o + NMM],
                                     start=(t == 0), stop=(t == 8))
                oA = opool.tile([COUT, NMM], F32, tag="oA")
                oB = opool.tile([COUT, NMM], F32, tag="oB")
                nc.vector.tensor_scalar_add(oA, psA, bsb[:, 0:1])
                nc.vector.tensor_scalar_add(oB, psB, bsb[:, 1:2])
                nc.sync.dma_start(out=y2p[0, :, g, :], in_=oA)
                nc.sync.dma_start(out=y2p[1, :, g, :], in_=oB)
    nc.compile()
    return nc


def get_program():
    if "nc" not in _PROGRAM_CACHE:
        _PROGRAM_CACHE["nc"] = _build_program()
    return _PROGRAM_CACHE["nc"]


def _pad_x(xpair):
    """[2, CIN, H, W] -> [128, XFLAT] width-65 padded flat layout."""
    out = np.zeros((2, CIN, NROW, PW), np.float32)
    out[:, :, 1:H + 1, 1:] = xpair
    return out.reshape(2 * CIN, XFLAT)


def make_in_maps(x, Wc, bc):
    """Per-core input maps: 2 samples per core."""
    in_maps = []
    for c in range(N_CORES):
        s0, s1 = 2 * c, 2 * c + 1
        # wct[ci + 64*s, t*64 + co] = Wc[sample, co, ci, t]
        w0 = Wc[s0].reshape(COUT, CIN, 9).transpose(1, 2, 0)
        w1 = Wc[s1].reshape(COUT, CIN, 9).transpose(1, 2, 0)
        wctm = np.concatenate([w0, w1], axis=0).reshape(128, 9 * COUT)
        bias2 = np.stack([bc[s0], bc[s1]], axis=1)  # [COUT, 2]
        in_maps.append({
            "xp": _pad_x(x[s0:s1 + 1]),
            "wct": np.ascontiguousarray(wctm),
            "bias2": np.ascontiguousarray(bias2),
        })
    return in_maps


# compaction index: y[.., h, w] = y2p[.., GIDX[h], JIDX[h, w]]
GIDX = np.minimum(np.arange(H) // RB, len(CHUNK_STARTS) - 1)
GIDX[H - 1] = len(CHUNK_STARTS) - 1
_h = np.arange(H)
_r0 = np.asarray(CHUNK_STARTS)[GIDX]
JIDX = (_h - _r0)[:, None] * PW + np.arange(W_SP)[None, :]


def gather_y(results):
    y = np.empty((B, COUT, H, W_SP), np.float32)
    for c in range(N_CORES):
        yp = results[c]["y2p"]  # [2, COUT, 10, NMM]
        y[2 * c:2 * c + 2] = yp[:, :, GIDX[:, None], JIDX]
    return y


def kernel(**inputs):
    x = np.asarray(inputs["x"], dtype=np.float32)
    w_gate = np.asarray(inputs["w_gate"], dtype=np.float32)
    W = np.asarray(inputs["W"], dtype=np.float32)
    b = np.asarray(inputs["b"], dtype=np.float32)
    # train is eval-only in the reference; the noise branch never runs.

    gates = _routing_gates(x, w_gate)
    loss = _aux_loss(gates)
    Wc = np.tensordot(gates, W.reshape(E, -1), axes=(1, 0)) \
        .reshape(B, COUT, CIN, 3, 3)
    bc = gates @ b  # [B, COUT]

    nc = get_program()
    res = run_bass_kernel_spmd(nc, make_in_maps(x, Wc, bc),
                               core_ids=list(range(N_CORES)))
    return (gather_y(res.results), loss)


# revision 3
# speedup vs baseline: 2.4092x; 1.1104x over previous
"""MoE gated 3x3 conv (eval path) on 8 trn2 NeuronCores.

Strategy:
- Routing (tiny: [16,64]@[64,16] -> softmax -> top-4 gates) and the scalar
  aux loss are computed on host; the gates determine a per-sample merged
  conv weight  Wc[b] = sum_e gates[b,e] * W[e]  (conv is linear in the
  weights, and zero-gate experts contribute nothing), so the device does
  exactly one 3x3 conv per sample instead of num_experts of them.
- Data parallel over batch: 2 samples per core. Sample 0 lives on SBUF
  partitions 0-63, sample 1 on partitions 64-127, so the two per-sample
  matmul chains are row-tiled (tile_position (0,0)/(64,0)) and the PE
  runs both concurrently.
- Width-65 padded image layout: row i of the padded image is
  [0, x[i-1, 0..63]], with zero rows above and below. A single zero
  column between consecutive rows serves as BOTH the right pad of row i
  and the left pad of row i+1, so every conv tap (dy, dx) is a pure flat
  shift by dy*65+dx and the matmul moving operand is fully contiguous.
- The conv is 9 shift-matmuls accumulating in PSUM over flat 512-wide
  windows (not row-aligned; junk columns are stripped on host). 9 windows
  cover the 64x65 flat output space.
- Matmuls run as float32r (TF32-like single-pass PE mode, ~1.5e-4
  scale-relative output error); PSUM accumulation stays fp32.
- x is loaded in 5 range-pieces so early windows' matmuls start while the
  rest of the image is still in flight.
"""

import numpy as np

import concourse.bacc as bacc
import concourse.tile as tile
from concourse import mybir
from concourse.bass_utils import run_bass_kernel_spmd

N_CORES = 8
B, CIN, COUT, E = 16, 64, 64, 16
H = W_SP = 64
KTOP = 4
PW = W_SP + 1          # 65: one shared zero column per row
NROW = 73              # padded rows (top zero, data, bottom zero + overrun)
XFLAT = NROW * PW      # 4745
NMM = 512              # flat window width per PSUM accumulation group
NWIN = 9               # ceil(64*65 / 512)
XPIECES = (768, 1792, 2816, 3840, XFLAT)  # x-load split points (flat, excl.)
F32 = mybir.dt.float32
MM_DT = mybir.dt.float32r

_PROGRAM_CACHE = {}


def _routing_gates(x, w_gate):
    """Eval-path gates, mirroring the reference: softmax over clean logits,
    top-4 renormalized. [B, E] float32."""
    gate_x = x.reshape(B, CIN, H * W_SP).mean(axis=2)      # [B, Cin]
    logits = gate_x.astype(np.float32) @ w_gate            # [B, E]
    m = logits.max(axis=1, keepdims=True)
    ex = np.exp(logits - m)
    sm = ex / ex.sum(axis=1, keepdims=True)
    idx = np.argsort(-sm, axis=1, kind="stable")[:, :KTOP]
    vals = np.take_along_axis(sm, idx, axis=1)
    gk = vals / (vals.sum(axis=1, keepdims=True) + 1e-6)
    gates = np.zeros((B, E), np.float32)
    np.put_along_axis(gates, idx, gk.astype(np.float32), axis=1)
    return gates


def _aux_loss(gates):
    load = (gates > 0).sum(axis=0).astype(np.float32)
    importance = gates.sum(axis=0).astype(np.float32)

    def cv_sq(v):
        return v.var(ddof=1) / (v.mean() ** 2 + 1e-10)

    return np.float32((cv_sq(importance) + cv_sq(load)) * 0.01)


def _build_program():
    nc = bacc.Bacc("TRN2", target_bir_lowering=False, debug=False,
                   num_devices=N_CORES)
    xp = nc.dram_tensor("xp", [128, XFLAT], MM_DT, kind="ExternalInput").ap()
    wct = nc.dram_tensor("wct", [128, 9 * COUT], MM_DT,
                         kind="ExternalInput").ap()
    bias2 = nc.dram_tensor("bias2", [COUT, 2], F32, kind="ExternalInput").ap()
    y2p = nc.dram_tensor("y2p", [2, COUT, NWIN, NMM], F32,
                         kind="ExternalOutput").ap()

    with tile.TileContext(nc) as tc:
        with tc.tile_pool(name="xs", bufs=1) as xpool, \
             tc.tile_pool(name="w", bufs=1) as wpool, \
             tc.tile_pool(name="out", bufs=3) as opool, \
             tc.tile_pool(name="ps", bufs=2, space="PSUM") as pspool:
            wsb = wpool.tile([128, 9 * COUT], MM_DT)
            nc.sync.dma_start(out=wsb, in_=wct)
            w3 = wsb.rearrange("p (t c) -> p t c", t=9)
            bsb = wpool.tile([COUT, 2], F32)
            nc.sync.dma_start(out=bsb, in_=bias2)
            xs = xpool.tile([128, XFLAT], MM_DT)
            lo = 0
            for hi in XPIECES:
                nc.sync.dma_start(out=xs[:, lo:hi], in_=xp[:, lo:hi])
                lo = hi

            for g in range(NWIN):
                psA = pspool.tile([COUT, NMM], F32, tag="psA")
                psB = pspool.tile([COUT, NMM], F32, tag="psB")
                for t in range(9):
                    dy, dx = divmod(t, 3)
                    o = g * NMM + dy * PW + dx
                    nc.tensor.matmul(psA, lhsT=w3[0:64, t, :],
                                     rhs=xs[0:64, o:o + NMM],
                                     start=(t == 0), stop=(t == 8))
                    nc.tensor.matmul(psB, lhsT=w3[64:128, t, :],
                                     rhs=xs[64:128, o:o + NMM],
                                     start=(t == 0), stop=(t == 8))
                oA = opool.tile([COUT, NMM], F32, tag="oA")
                oB = opool.tile([COUT, NMM], F32, tag="oB")
                nc.vector.tensor_scalar_add(oA, psA, bsb[:, 0:1])
                nc.vector.tensor_scalar_add(oB, psB, bsb[:, 1:2])
                nc.sync.dma_start(out=y2p[0, :, g, :], in_=oA)
                nc.sync.dma_start(out=y2p[1, :, g, :], in_=oB)
    nc.compile()
    return nc


def get_program():
    if "nc" not in _PROGRAM_CACHE:
        _PROGRAM_CACHE["nc"] = _build_program()
    return _PROGRAM_CACHE["nc"]


def _pad_x(xpair):
    """[2, CIN, H, W] -> [128, XFLAT] width-65 padded flat layout."""
    out = np.zeros((2, CIN, NROW, PW), np.float32)
    out[:, :, 1:H + 1, 1:] = xpair
    return out.reshape(2 * CIN, XFLAT)


def make_in_maps(x, Wc, bc):
    """Per-core input maps: 2 samples per core."""
    in_maps = []
    for c in range(N_CORES):
        s0, s1 = 2 * c, 2 * c + 1
        # wct[ci + 64*s, t*64 + co] = Wc[sample, co, ci, t]
        w0 = Wc[s0].reshape(COUT, CIN, 9).transpose(1, 2, 0)
        w1 = Wc[s1].reshape(COUT, CIN, 9).transpose(1, 2, 0)
        wctm = np.concatenate([w0, w1], axis=0).reshape(128, 9 * COUT)
        bias2 = np.stack([bc[s0], bc[s1]], axis=1)  # [COUT, 2]
        in_maps.append({
            "xp": _pad_x(x[s0:s1 + 1]),
            "wct": np.ascontiguousarray(wctm),
            "bias2": np.ascontiguousarray(bias2),
        })
    return in_maps


# compaction index: y[.., h, w] = y2p[.., GIDX[h, w], JIDX[h, w]]
_f = np.arange(H)[:, None] * PW + np.arange(W_SP)[None, :]
GIDX = _f // NMM
JIDX = _f % NMM


def gather_y(results):
    y = np.empty((B, COUT, H, W_SP), np.float32)
    for c in range(N_CORES):
        yp = results[c]["y2p"]  # [2, COUT, NWIN, NMM]
        y[2 * c:2 * c + 2] = yp[:, :, GIDX, JIDX]
    return y


def kernel(**inputs):
    x = np.asarray(inputs["x"], dtype=np.float32)
    w_gate = np.asarray(inputs["w_gate"], dtype=np.float32)
    W = np.asarray(inputs["W"], dtype=np.float32)
    b = np.asarray(inputs["b"], dtype=np.float32)
    # train is eval-only in the reference; the noise branch never runs.

    gates = _routing_gates(x, w_gate)
    loss = _aux_loss(gates)
    Wc = np.tensordot(gates, W.reshape(E, -1), axes=(1, 0)) \
        .reshape(B, COUT, CIN, 3, 3)
    bc = gates @ b  # [B, COUT]

    nc = get_program()
    res = run_bass_kernel_spmd(nc, make_in_maps(x, Wc, bc),
                               core_ids=list(range(N_CORES)))
    return (gather_y(res.results), loss)
